# revision 3
# baseline (speedup 1.0000x reference)
"""MetaRoPE kernel for Trainium2, 8 NeuronCores.

Reference computation:
    r = rotate_m[token_positions]            # [S, D, D], block-diag 2x2 rotations
    out = einsum('bhsi,soi->bhso', x, r)     # x: [4, 32, 4096, 64] fp32

Because r is block-diagonal with 2x2 blocks, for each position s and pair k:
    out[2k]   = a*x[2k] + b*x[2k+1]     (a = r[2k,2k],   b = r[2k,2k+1])
    out[2k+1] = c*x[2k+1] + d*x[2k]     (c = r[2k+1,2k+1], d = r[2k+1,2k])
which we compute elementwise as
    out = x * A + pairswap(x * B')
with host-precomputed tables A, B' of shape [S, D]:
    A[s,2k] = a, A[s,2k+1] = c
    B'[s,2k] = d, B'[s,2k+1] = b       (B' is pre-pairswapped so that
                                        pairswap(x*B') lands b*x_odd on even
                                        lanes and d*x_even on odd lanes)

Sharding: x reshaped to [128 (b,h) slabs, 4096, 64]; 16 slabs per core.
Each slab [4096*64] is viewed as [128 partitions, 2048 free] (contiguous per
partition). Tables are replicated to every core as [128, 2048] tiles that
match that layout for every slab.

Per core: 4 chunks of 4 slabs. Each chunk: one 4 MiB load (HWDGE on sync),
per-slab DVE tensor_mul x2 + pair-swapped add, one 4 MiB store (HWDGE on
scalar ring).
"""

import sys

import numpy as np

_TRN_REPO = "/opt/trn_rl_repo"
if _TRN_REPO not in sys.path:
    sys.path.insert(0, _TRN_REPO)

B, H, S, D = 4, 32, 4096, 64
BH = B * H                      # 128 (b,h) slabs
N_CORES = 8
BH_PER_CORE = BH // N_CORES     # 16 slabs per core
FREE = (S // 128) * D           # 2048 free elements per partition per slab
ROWS = BH_PER_CORE * 128        # 2048 dram rows per core, [ROWS, FREE] fp32
CHUNK = 4                       # slabs per DMA chunk (4 MiB transfers)

_prog_cache = {}


def _build_program():
    """Build (and cache) the SPMD Bass program for one core."""
    if "nc" in _prog_cache:
        return _prog_cache["nc"]

    import concourse.bacc as bacc
    import concourse.mybir as mybir
    import concourse.tile as tile

    f32 = mybir.dt.float32
    nc = bacc.Bacc(
        "TRN2", target_bir_lowering=False, debug=False, num_devices=N_CORES
    )
    x_d = nc.dram_tensor("x", [ROWS, FREE], f32, kind="ExternalInput").ap()
    ta_d = nc.dram_tensor("ta", [128, FREE], f32, kind="ExternalInput").ap()
    tb_d = nc.dram_tensor("tb", [128, FREE], f32, kind="ExternalInput").ap()
    o_d = nc.dram_tensor("out", [ROWS, FREE], f32, kind="ExternalOutput").ap()

    n_chunks = BH_PER_CORE // CHUNK
    cfree = CHUNK * FREE

    with tile.TileContext(nc) as tc:
        with (
            tc.tile_pool(name="tabs", bufs=1) as tabs,
            tc.tile_pool(name="xin", bufs=2) as xin,
            tc.tile_pool(name="u", bufs=3) as upool,
            tc.tile_pool(name="o", bufs=2) as opool,
        ):
            ta = tabs.tile([128, FREE], f32)
            nc.sync.dma_start(ta[:], ta_d[:])
            tb = tabs.tile([128, FREE], f32)
            nc.sync.dma_start(tb[:], tb_d[:])

            for ci in range(n_chunks):
                rows = x_d[ci * CHUNK * 128 : (ci + 1) * CHUNK * 128, :]
                src = rows.rearrange("(j p) f -> p j f", j=CHUNK)
                xt = xin.tile([128, cfree], f32)
                nc.sync.dma_start(
                    xt[:].rearrange("p (j f) -> p j f", j=CHUNK), src
                )

                ot = opool.tile([128, cfree], f32)
                for j in range(CHUNK):
                    xs = xt[:, j * FREE : (j + 1) * FREE]
                    os_ = ot[:, j * FREE : (j + 1) * FREE]
                    u = upool.tile([128, FREE], f32)
                    nc.vector.tensor_mul(u[:], xs, tb[:])
                    nc.vector.tensor_mul(os_, xs, ta[:])
                    usw = u[:].rearrange("p (n two) -> p n two", two=2)[:, :, ::-1]
                    os3 = os_.rearrange("p (n two) -> p n two", two=2)
                    nc.vector.tensor_add(os3, os3, usw)

                orows = o_d[ci * CHUNK * 128 : (ci + 1) * CHUNK * 128, :]
                dst = orows.rearrange("(j p) f -> p j f", j=CHUNK)
                nc.scalar.dma_start(
                    dst, ot[:].rearrange("p (j f) -> p j f", j=CHUNK)
                )

    nc.compile()
    _prog_cache["nc"] = nc
    return nc


def _tables(token_positions, rotate_m):
    """Host-precompute the [128, FREE] A and B' tables (see module docstring)."""
    r = np.asarray(rotate_m, dtype=np.float32)[np.asarray(token_positions)]
    idx = np.arange(D // 2) * 2
    a = r[:, idx, idx]            # x_even -> out_even
    b = r[:, idx, idx + 1]        # x_odd  -> out_even
    c = r[:, idx + 1, idx + 1]    # x_odd  -> out_odd
    d = r[:, idx + 1, idx]        # x_even -> out_odd
    A = np.empty((S, D), np.float32)
    A[:, 0::2] = a
    A[:, 1::2] = c
    Bp = np.empty((S, D), np.float32)
    Bp[:, 0::2] = d
    Bp[:, 1::2] = b
    return (
        np.ascontiguousarray(A.reshape(128, FREE)),
        np.ascontiguousarray(Bp.reshape(128, FREE)),
    )


def _in_maps(x, token_positions, rotate_m):
    ta, tb = _tables(token_positions, rotate_m)
    xs = np.ascontiguousarray(np.asarray(x, dtype=np.float32)).reshape(
        N_CORES, ROWS, FREE
    )
    return [{"x": xs[i], "ta": ta, "tb": tb} for i in range(N_CORES)]


def _run(x, token_positions, rotate_m, trace=False, trace_cores=None):
    from concourse.bass_utils import run_bass_kernel_spmd

    nc = _build_program()
    in_maps = _in_maps(x, token_positions, rotate_m)
    res = run_bass_kernel_spmd(
        nc,
        in_maps,
        list(range(N_CORES)),
        trace=trace,
        trace_cores=trace_cores,
    )
    out = np.concatenate(
        [res.results[i]["out"].reshape(1, ROWS * FREE) for i in range(N_CORES)]
    ).reshape(B, H, S, D)
    return out, res


def kernel(x, token_positions, rotate_m, **_unused):
    out, _ = _run(x, token_positions, rotate_m, trace=False)
    return out
